# revision 16
# baseline (speedup 1.0000x reference)
"""Butterworth bandpass filter (order-8 IIR, 9-tap b/a) over x[16, 64, 65536].

Strategy: the filter's slowest pole has |p| = 0.966, so the impulse response
decays to ~6e-5 (l2) after 256 samples -- far below the ~5.5e-3 noise the f32
reference recurrence itself accumulates vs exact arithmetic. The IIR is
therefore computed as a 256-tap causal FIR, which maps onto the TensorEngine
as a banded block-Toeplitz matmul:

  - 1024 signals sharded 128-per-core across 8 NeuronCores (data parallel).
  - Per core, x[128, 65536] is processed in output windows of 512 timesteps.
  - x is loaded via casting SWDGE DMAs straight into float32r tiles; input
    blocks [128 sig, 128 t] are PE-transposed (1.5 cyc/row for f32r) to
    [t, sig]; each window's output y[sig, 512] = sum_p XT_q(p).T @ slab_p
    where slab_p[i, n] = h[n + W - 128 p - i] is a [128, width] slice of the
    Toeplitz band (widths >= 256 so f32r matmuls stream at 1 cycle/row).
  - Windows accumulate in one PSUM bank (6 matmuls; the first covers columns
    [0, 384) with start=True, clearing the bank's has_written zero-region).
  - Measured ~200 us/core on TRN2, vs a ~179 us HBM roofline (64 MiB/core
    round trip at ~358 GB/s); rel err 5.4e-3 = the f32 reference noise floor.
"""

import os
from contextlib import ExitStack

import numpy as np

B, C, T = 16, 64, 65536
NSIG = B * C              # 1024 signals
N_CORES = 8
SIG_PER_CORE = NSIG // N_CORES  # 128

W = 256                   # FIR taps (l2 tail ~6e-5, far below the ~5.5e-3 f32 ref noise)
WIN = 512                 # output window (one PSUM bank of f32)
NWIN = T // WIN           # 128
CHUNK = 1024              # input DMA chunk (512 KiB)
NCHUNK = T // CHUNK       # 64
WPC = CHUNK // WIN        # windows per input chunk = 2
OUT_CHUNK = 1024          # output DMA chunk (512 KiB)
WPO = OUT_CHUNK // WIN    # windows per output chunk = 2

# Per contributor p (input block q = 4J - 2 + p): output column range written.
# Widths are >= 256 so float32r matmuls run at 1 cycle/row.
SLAB_SPECS = [(0, 256), (0, 256), (0, 384), (128, 384), (256, 256), (256, 256)]
SLAB_OFFS = np.cumsum([0] + [w for _, w in SLAB_SPECS]).tolist()
SLAB_COLS = SLAB_OFFS[-1]  # 2304
# p = 2 ([0,384)) runs first with start=True (clears the whole PSUM bank's
# has_written zero-region); p = 3 ([128,512)) covers the remaining columns.
EXEC_ORDER = [2, 3, 0, 1, 4, 5]

_NC_CACHE = {}


def _build_nc():
    import concourse.bacc as bacc
    import concourse.tile as tile
    from concourse import mybir

    f32 = mybir.dt.float32
    f32r = mybir.dt.float32r

    nc = bacc.Bacc("TRN2", target_bir_lowering=False, debug=False)
    x_d = nc.dram_tensor("x", [SIG_PER_CORE, T], f32, kind="ExternalInput")
    slab_d = nc.dram_tensor("slabs", [128, SLAB_COLS], f32, kind="ExternalInput")
    ident_d = nc.dram_tensor("ident", [128, 128], f32, kind="ExternalInput")
    y_d = nc.dram_tensor("y", [SIG_PER_CORE, T], f32, kind="ExternalOutput")

    with tile.TileContext(nc) as tc, ExitStack() as ctx:
        const = ctx.enter_context(tc.tile_pool(name="const", bufs=1))
        inpool = ctx.enter_context(tc.tile_pool(name="inpool", bufs=10))
        xtpool = ctx.enter_context(tc.tile_pool(name="xtpool", bufs=6))
        outpool = ctx.enter_context(tc.tile_pool(name="outpool", bufs=10))
        pst = ctx.enter_context(tc.tile_pool(name="pst", bufs=3, space="PSUM"))
        psy = ctx.enter_context(tc.tile_pool(name="psy", bufs=5, space="PSUM"))

        in_tiles = {}
        # Prefetch the first two x chunks before anything else on the SWDGE
        # path so the PE pipeline starts as early as possible.
        for g0 in range(2):
            t_in = inpool.tile([SIG_PER_CORE, CHUNK], f32r, tag="in")
            nc.gpsimd.dma_start(t_in[:], x_d.ap()[:, g0 * CHUNK:(g0 + 1) * CHUNK])
            in_tiles[g0] = t_in

        ident = const.tile([128, 128], f32r)
        nc.gpsimd.dma_start(ident[:], ident_d.ap()[:])
        slab_f = const.tile([128, SLAB_COLS], f32)
        nc.sync.dma_start(slab_f[:], slab_d.ap()[:])
        slab = const.tile([128, SLAB_COLS], f32r)
        nc.vector.tensor_copy(slab[:], slab_f[:])

        xt_chunks = {}
        out_tile = None
        for J in range(NWIN):
            g = J // WPC
            if J % WPC == 0 and g not in in_tiles:
                t_in = inpool.tile([SIG_PER_CORE, CHUNK], f32r, tag="in")
                nc.gpsimd.dma_start(t_in[:], x_d.ap()[:, g * CHUNK:(g + 1) * CHUNK])
                in_tiles[g] = t_in

            # Transpose the 4 input blocks of window J: [sig, t] -> [t, sig].
            ps_tr = pst.tile([128, 512], f32r, tag="ps_tr")
            xin = in_tiles[g]
            base = (J % WPC) * WIN
            for c in range(4):
                nc.tensor.transpose(
                    ps_tr[:, c * 128:(c + 1) * 128],
                    xin[:, base + c * 128: base + (c + 1) * 128],
                    ident[:],
                )
            xt = xtpool.tile([128, 512], f32r, tag="xt")
            nc.vector.tensor_copy(xt[:], ps_tr[:])
            xt_chunks[J] = xt
            if J - 2 in xt_chunks:
                del xt_chunks[J - 2]

            # FIR window J: accumulate 6 banded-Toeplitz matmuls into one bank.
            ps_y = psy.tile([128, WIN], f32, tag="ps_y")
            first = True
            for p in EXEC_ORDER:
                q = 4 * J - 2 + p
                if q < 0:
                    continue
                cj, cc = divmod(q, 4)
                lhsT = xt_chunks[cj][:, cc * 128:(cc + 1) * 128]
                c0, w = SLAB_SPECS[p]
                off = SLAB_OFFS[p]
                nc.tensor.matmul(
                    ps_y[:, c0:c0 + w],
                    lhsT,
                    slab[:, off:off + w],
                    start=first,
                    stop=(p == EXEC_ORDER[-1]),
                )
                first = False

            if J % WPO == 0:
                out_tile = outpool.tile([SIG_PER_CORE, OUT_CHUNK], f32, tag="out")
            nc.vector.tensor_copy(
                out_tile[:, (J % WPO) * WIN:(J % WPO + 1) * WIN], ps_y[:]
            )
            if J % WPO == WPO - 1:
                nc.sync.dma_start(
                    y_d.ap()[:, (J // WPO) * OUT_CHUNK:(J // WPO + 1) * OUT_CHUNK],
                    out_tile[:],
                )
    nc.compile()
    return nc


def _get_nc():
    if "nc" not in _NC_CACHE:
        _NC_CACHE["nc"] = _build_nc()
    return _NC_CACHE["nc"]


def _impulse_response(b, a, n):
    b = np.asarray(b, np.float64)
    a = np.asarray(a, np.float64)
    b = b / a[0]
    a = a / a[0]
    h = np.zeros(n, np.float64)
    for t in range(n):
        acc = b[t] if t < len(b) else 0.0
        kmax = min(len(a) - 1, t)
        for k in range(1, kmax + 1):
            acc -= a[k] * h[t - k]
        h[t] = acc
    return h


def _build_slabs(h):
    """slab_p[i, j] = h[(c0_p + j) + W - 128 p - i], zero outside [0, W)."""
    i = np.arange(128)
    slabs = np.zeros((128, SLAB_COLS), np.float32)
    for p, ((c0, w), off) in enumerate(zip(SLAB_SPECS, SLAB_OFFS)):
        n = c0 + np.arange(w)
        d = n[None, :] + W - 128 * p - i[:, None]
        valid = (d >= 0) & (d < W)
        vals = np.where(valid, h[np.clip(d, 0, W - 1)], 0.0)
        slabs[:, off:off + w] = vals.astype(np.float32)
    return slabs


def kernel_with_results(x, b, a, trace=False):
    from concourse.bass_utils import run_bass_kernel_spmd

    x = np.asarray(x, np.float32)
    h = _impulse_response(np.asarray(b), np.asarray(a), W)
    slabs = _build_slabs(h)
    ident = np.eye(128, dtype=np.float32)

    xs = np.asarray(x).reshape(NSIG, T)
    in_maps = [
        {
            "x": np.ascontiguousarray(xs[c * SIG_PER_CORE:(c + 1) * SIG_PER_CORE]),
            "slabs": slabs,
            "ident": ident,
        }
        for c in range(N_CORES)
    ]
    nc = _get_nc()
    res = run_bass_kernel_spmd(nc, in_maps, core_ids=list(range(N_CORES)), trace=trace)
    y = np.concatenate([res.results[c]["y"] for c in range(N_CORES)], axis=0)
    return y.reshape(B, C, T).astype(np.float32), res


def kernel(x, b, a):
    os.environ.setdefault("BASS_NEVER_TRACE", "1")
    y, _ = kernel_with_results(x, b, a, trace=False)
    return y



# revision 17
# speedup vs baseline: 1.0610x; 1.0610x over previous
"""Butterworth bandpass filter (order-8 IIR, 9-tap b/a) over x[16, 64, 65536].

Strategy: the filter's slowest pole has |p| = 0.966, so the impulse response
decays to ~6e-5 (l2) after 256 samples -- far below the ~5.5e-3 noise the f32
reference recurrence itself accumulates vs exact arithmetic. The IIR is
therefore computed as a 256-tap causal FIR, which maps onto the TensorEngine
as a banded block-Toeplitz matmul:

  - 1024 signals sharded 128-per-core across 8 NeuronCores (data parallel).
  - Per core, x[128, 65536] is processed in output windows of 512 timesteps.
  - x is loaded via casting SWDGE DMAs straight into float32r tiles; input
    blocks [128 sig, 128 t] are PE-transposed (1.5 cyc/row for f32r) to
    [t, sig]; each window's output y[sig, 512] = sum_p XT_q(p).T @ slab_p
    where slab_p[i, n] = h[n + W - 128 p - i] is a [128, width] slice of the
    Toeplitz band (widths >= 256 so f32r matmuls stream at 1 cycle/row).
  - Windows accumulate in one PSUM bank (6 matmuls; the first covers columns
    [0, 384) with start=True, clearing the bank's has_written zero-region).
  - Measured ~200 us/core on TRN2, vs a ~179 us HBM roofline (64 MiB/core
    round trip at ~358 GB/s); rel err 5.4e-3 = the f32 reference noise floor.
"""

import os
from contextlib import ExitStack

import numpy as np

B, C, T = 16, 64, 65536
NSIG = B * C              # 1024 signals
N_CORES = 8
SIG_PER_CORE = NSIG // N_CORES  # 128

W = 256                   # FIR taps (l2 tail ~6e-5, far below the ~5.5e-3 f32 ref noise)
WIN = 512                 # output window (one PSUM bank of f32)
NWIN = T // WIN           # 128
CHUNK = 1024              # input DMA chunk (512 KiB)
NCHUNK = T // CHUNK       # 64
WPC = CHUNK // WIN        # windows per input chunk = 2
OUT_CHUNK = 1024          # output DMA chunk (512 KiB)
WPO = OUT_CHUNK // WIN    # windows per output chunk = 2

# Per contributor p (input block q = 4J - 2 + p): output column range written.
# Widths are >= 256 so float32r matmuls run at 1 cycle/row.
SLAB_SPECS = [(0, 256), (0, 256), (0, 384), (128, 384), (256, 256), (256, 256)]
SLAB_OFFS = np.cumsum([0] + [w for _, w in SLAB_SPECS]).tolist()
SLAB_COLS = SLAB_OFFS[-1]  # 2304
# p = 2 ([0,384)) runs first with start=True (clears the whole PSUM bank's
# has_written zero-region); p = 3 ([128,512)) covers the remaining columns.
EXEC_ORDER = [2, 3, 0, 1, 4, 5]

_NC_CACHE = {}


def _build_nc():
    import concourse.bacc as bacc
    import concourse.tile as tile
    from concourse import mybir

    f32 = mybir.dt.float32
    f32r = mybir.dt.float32r

    nc = bacc.Bacc("TRN2", target_bir_lowering=False, debug=False)
    x_d = nc.dram_tensor("x", [SIG_PER_CORE, T], f32, kind="ExternalInput")
    slab_d = nc.dram_tensor("slabs", [128, SLAB_COLS], f32, kind="ExternalInput")
    ident_d = nc.dram_tensor("ident", [128, 128], f32, kind="ExternalInput")
    y_d = nc.dram_tensor("y", [SIG_PER_CORE, T], f32, kind="ExternalOutput")

    with tile.TileContext(nc) as tc, ExitStack() as ctx:
        const = ctx.enter_context(tc.tile_pool(name="const", bufs=1))
        inpool = ctx.enter_context(tc.tile_pool(name="inpool", bufs=10))
        xtpool = ctx.enter_context(tc.tile_pool(name="xtpool", bufs=6))
        outpool = ctx.enter_context(tc.tile_pool(name="outpool", bufs=10))
        pst = ctx.enter_context(tc.tile_pool(name="pst", bufs=4, space="PSUM"))
        psy = ctx.enter_context(tc.tile_pool(name="psy", bufs=4, space="PSUM"))

        in_tiles = {}
        # Prefetch the first two x chunks before anything else on the SWDGE
        # path so the PE pipeline starts as early as possible.
        for g0 in range(2):
            t_in = inpool.tile([SIG_PER_CORE, CHUNK], f32r, tag="in")
            nc.gpsimd.dma_start(t_in[:], x_d.ap()[:, g0 * CHUNK:(g0 + 1) * CHUNK])
            in_tiles[g0] = t_in

        ident = const.tile([128, 128], f32r)
        nc.gpsimd.dma_start(ident[:], ident_d.ap()[:])
        slab_f = const.tile([128, SLAB_COLS], f32)
        nc.sync.dma_start(slab_f[:], slab_d.ap()[:])
        slab = const.tile([128, SLAB_COLS], f32r)
        nc.vector.tensor_copy(slab[:], slab_f[:])

        xt_chunks = {}
        out_tile = None
        for J in range(NWIN):
            g = J // WPC
            if J % WPC == 0 and g not in in_tiles:
                t_in = inpool.tile([SIG_PER_CORE, CHUNK], f32r, tag="in")
                nc.gpsimd.dma_start(t_in[:], x_d.ap()[:, g * CHUNK:(g + 1) * CHUNK])
                in_tiles[g] = t_in

            # Transpose the 4 input blocks of window J: [sig, t] -> [t, sig].
            ps_tr = pst.tile([128, 512], f32r, tag="ps_tr")
            xin = in_tiles[g]
            base = (J % WPC) * WIN
            for c in range(4):
                nc.tensor.transpose(
                    ps_tr[:, c * 128:(c + 1) * 128],
                    xin[:, base + c * 128: base + (c + 1) * 128],
                    ident[:],
                )
            xt = xtpool.tile([128, 512], f32r, tag="xt")
            nc.vector.tensor_copy(xt[:], ps_tr[:])
            xt_chunks[J] = xt
            if J - 2 in xt_chunks:
                del xt_chunks[J - 2]

            # FIR window J: accumulate 6 banded-Toeplitz matmuls into one bank.
            ps_y = psy.tile([128, WIN], f32, tag="ps_y")
            first = True
            for p in EXEC_ORDER:
                q = 4 * J - 2 + p
                if q < 0:
                    continue
                cj, cc = divmod(q, 4)
                lhsT = xt_chunks[cj][:, cc * 128:(cc + 1) * 128]
                c0, w = SLAB_SPECS[p]
                off = SLAB_OFFS[p]
                nc.tensor.matmul(
                    ps_y[:, c0:c0 + w],
                    lhsT,
                    slab[:, off:off + w],
                    start=first,
                    stop=(p == EXEC_ORDER[-1]),
                )
                first = False

            if J % WPO == 0:
                out_tile = outpool.tile([SIG_PER_CORE, OUT_CHUNK], f32, tag="out")
            nc.vector.tensor_copy(
                out_tile[:, (J % WPO) * WIN:(J % WPO + 1) * WIN], ps_y[:]
            )
            if J % WPO == WPO - 1:
                nc.sync.dma_start(
                    y_d.ap()[:, (J // WPO) * OUT_CHUNK:(J // WPO + 1) * OUT_CHUNK],
                    out_tile[:],
                )
    nc.compile()
    return nc


def _get_nc():
    if "nc" not in _NC_CACHE:
        _NC_CACHE["nc"] = _build_nc()
    return _NC_CACHE["nc"]


def _impulse_response(b, a, n):
    b = np.asarray(b, np.float64)
    a = np.asarray(a, np.float64)
    b = b / a[0]
    a = a / a[0]
    h = np.zeros(n, np.float64)
    for t in range(n):
        acc = b[t] if t < len(b) else 0.0
        kmax = min(len(a) - 1, t)
        for k in range(1, kmax + 1):
            acc -= a[k] * h[t - k]
        h[t] = acc
    return h


def _build_slabs(h):
    """slab_p[i, j] = h[(c0_p + j) + W - 128 p - i], zero outside [0, W)."""
    i = np.arange(128)
    slabs = np.zeros((128, SLAB_COLS), np.float32)
    for p, ((c0, w), off) in enumerate(zip(SLAB_SPECS, SLAB_OFFS)):
        n = c0 + np.arange(w)
        d = n[None, :] + W - 128 * p - i[:, None]
        valid = (d >= 0) & (d < W)
        vals = np.where(valid, h[np.clip(d, 0, W - 1)], 0.0)
        slabs[:, off:off + w] = vals.astype(np.float32)
    return slabs


def kernel_with_results(x, b, a, trace=False):
    from concourse.bass_utils import run_bass_kernel_spmd

    x = np.asarray(x, np.float32)
    h = _impulse_response(np.asarray(b), np.asarray(a), W)
    slabs = _build_slabs(h)
    ident = np.eye(128, dtype=np.float32)

    xs = np.asarray(x).reshape(NSIG, T)
    in_maps = [
        {
            "x": np.ascontiguousarray(xs[c * SIG_PER_CORE:(c + 1) * SIG_PER_CORE]),
            "slabs": slabs,
            "ident": ident,
        }
        for c in range(N_CORES)
    ]
    nc = _get_nc()
    res = run_bass_kernel_spmd(nc, in_maps, core_ids=list(range(N_CORES)), trace=trace)
    y = np.concatenate([res.results[c]["y"] for c in range(N_CORES)], axis=0)
    return y.reshape(B, C, T).astype(np.float32), res


def kernel(x, b, a):
    os.environ.setdefault("BASS_NEVER_TRACE", "1")
    y, _ = kernel_with_results(x, b, a, trace=False)
    return y



# revision 18
# speedup vs baseline: 1.0880x; 1.0254x over previous
"""Butterworth bandpass filter (order-8 IIR, 9-tap b/a) over x[16, 64, 65536].

Strategy: the filter's slowest pole has |p| = 0.966, so the impulse response
decays to ~6e-5 (l2) after 256 samples -- far below the ~5.5e-3 noise the f32
reference recurrence itself accumulates vs exact arithmetic. The IIR is
therefore computed as a 256-tap causal FIR, which maps onto the TensorEngine
as a banded block-Toeplitz matmul:

  - 1024 signals sharded 128-per-core across 8 NeuronCores (data parallel).
  - Per core, x[128, 65536] is processed in output windows of 512 timesteps.
  - x is loaded via casting SWDGE DMAs straight into float32r tiles; input
    blocks [128 sig, 128 t] are PE-transposed (1.5 cyc/row for f32r) to
    [t, sig]; each window's output y[sig, 512] = sum_p XT_q(p).T @ slab_p
    where slab_p[i, n] = h[n + W - 128 p - i] is a [128, width] slice of the
    Toeplitz band (widths >= 256 so f32r matmuls stream at 1 cycle/row).
  - Windows accumulate in one PSUM bank (6 matmuls; the first covers columns
    [0, 384) with start=True, clearing the bank's has_written zero-region).
  - Measured ~200 us/core on TRN2, vs a ~179 us HBM roofline (64 MiB/core
    round trip at ~358 GB/s); rel err 5.4e-3 = the f32 reference noise floor.
"""

import os
from contextlib import ExitStack

import numpy as np

B, C, T = 16, 64, 65536
NSIG = B * C              # 1024 signals
N_CORES = 8
SIG_PER_CORE = NSIG // N_CORES  # 128

W = 256                   # FIR taps (l2 tail ~6e-5, far below the ~5.5e-3 f32 ref noise)
WIN = 512                 # output window (one PSUM bank of f32)
NWIN = T // WIN           # 128
CHUNK = 1024              # input DMA chunk (512 KiB)
NCHUNK = T // CHUNK       # 64
WPC = CHUNK // WIN        # windows per input chunk = 2
OUT_CHUNK = 1024          # output DMA chunk (512 KiB)
WPO = OUT_CHUNK // WIN    # windows per output chunk = 2

# Per contributor p (input block q = 4J - 2 + p): output column range written.
# Widths are >= 256 so float32r matmuls run at 1 cycle/row.
SLAB_SPECS = [(0, 256), (0, 256), (0, 384), (128, 384), (256, 256), (256, 256)]
SLAB_OFFS = np.cumsum([0] + [w for _, w in SLAB_SPECS]).tolist()
SLAB_COLS = SLAB_OFFS[-1]  # 2304
# p = 2 ([0,384)) runs first with start=True (clears the whole PSUM bank's
# has_written zero-region); p = 3 ([128,512)) covers the remaining columns.
EXEC_ORDER = [2, 3, 0, 1, 4, 5]

_NC_CACHE = {}


def _build_nc():
    import concourse.bacc as bacc
    import concourse.tile as tile
    from concourse import mybir

    f32 = mybir.dt.float32
    f32r = mybir.dt.float32r

    nc = bacc.Bacc("TRN2", target_bir_lowering=False, debug=False)
    x_d = nc.dram_tensor("x", [SIG_PER_CORE, T], f32, kind="ExternalInput")
    slab_d = nc.dram_tensor("slabs", [128, SLAB_COLS], f32, kind="ExternalInput")
    ident_d = nc.dram_tensor("ident", [128, 128], f32, kind="ExternalInput")
    y_d = nc.dram_tensor("y", [SIG_PER_CORE, T], f32, kind="ExternalOutput")

    with tile.TileContext(nc) as tc, ExitStack() as ctx:
        const = ctx.enter_context(tc.tile_pool(name="const", bufs=1))
        inpool = ctx.enter_context(tc.tile_pool(name="inpool", bufs=14))
        xtpool = ctx.enter_context(tc.tile_pool(name="xtpool", bufs=6))
        outpool = ctx.enter_context(tc.tile_pool(name="outpool", bufs=10))
        pst = ctx.enter_context(tc.tile_pool(name="pst", bufs=4, space="PSUM"))
        psy = ctx.enter_context(tc.tile_pool(name="psy", bufs=4, space="PSUM"))

        in_tiles = {}
        # Prefetch the first two x chunks before anything else on the SWDGE
        # path so the PE pipeline starts as early as possible.
        for g0 in range(2):
            t_in = inpool.tile([SIG_PER_CORE, CHUNK], f32r, tag="in")
            nc.gpsimd.dma_start(t_in[:], x_d.ap()[:, g0 * CHUNK:(g0 + 1) * CHUNK])
            in_tiles[g0] = t_in

        ident = const.tile([128, 128], f32r)
        nc.gpsimd.dma_start(ident[:], ident_d.ap()[:])
        slab_f = const.tile([128, SLAB_COLS], f32)
        nc.sync.dma_start(slab_f[:], slab_d.ap()[:])
        slab = const.tile([128, SLAB_COLS], f32r)
        nc.vector.tensor_copy(slab[:], slab_f[:])

        xt_chunks = {}
        out_tile = None

        def load_chunk(g):
            if g in in_tiles or g >= NCHUNK:
                return
            t_in = inpool.tile([SIG_PER_CORE, CHUNK], f32r, tag="in")
            nc.gpsimd.dma_start(t_in[:], x_d.ap()[:, g * CHUNK:(g + 1) * CHUNK])
            in_tiles[g] = t_in

        def emit_transposes(J):
            # Transpose the 4 input blocks of window J: [sig, t] -> [t, sig].
            # Returns the 4 transpose emitters so they can be interleaved
            # between the previous window's matmuls (each transpose's internal
            # LDWEIGHTS then prefetches under a longer matmul stream).
            load_chunk(J // WPC)
            ps_tr = pst.tile([128, 512], f32r, tag="ps_tr")
            xin = in_tiles[J // WPC]
            base = (J % WPC) * WIN

            def one(c):
                nc.tensor.transpose(
                    ps_tr[:, c * 128:(c + 1) * 128],
                    xin[:, base + c * 128: base + (c + 1) * 128],
                    ident[:],
                )

            def finish():
                xt = xtpool.tile([128, 512], f32r, tag="xt")
                nc.vector.tensor_copy(xt[:], ps_tr[:])
                xt_chunks[J] = xt
                if J - 2 in xt_chunks:
                    del xt_chunks[J - 2]

            return one, finish

        one0, finish0 = emit_transposes(0)
        for c in range(4):
            one0(c)
        finish0()

        for J in range(NWIN):
            nxt = emit_transposes(J + 1) if J + 1 < NWIN else None

            # FIR window J: accumulate 6 banded-Toeplitz matmuls into one bank,
            # with window J+1's transposes interleaved between them.
            ps_y = psy.tile([128, WIN], f32, tag="ps_y")
            first = True
            for k, p in enumerate(EXEC_ORDER):
                q = 4 * J - 2 + p
                if q >= 0:
                    cj, cc = divmod(q, 4)
                    lhsT = xt_chunks[cj][:, cc * 128:(cc + 1) * 128]
                    c0, w = SLAB_SPECS[p]
                    off = SLAB_OFFS[p]
                    nc.tensor.matmul(
                        ps_y[:, c0:c0 + w],
                        lhsT,
                        slab[:, off:off + w],
                        start=first,
                        stop=(p == EXEC_ORDER[-1]),
                    )
                    first = False
                if nxt is not None and k < 4:
                    nxt[0](k)

            if J % WPO == 0:
                out_tile = outpool.tile([SIG_PER_CORE, OUT_CHUNK], f32, tag="out")
            nc.vector.tensor_copy(
                out_tile[:, (J % WPO) * WIN:(J % WPO + 1) * WIN], ps_y[:]
            )
            if J % WPO == WPO - 1:
                nc.sync.dma_start(
                    y_d.ap()[:, (J // WPO) * OUT_CHUNK:(J // WPO + 1) * OUT_CHUNK],
                    out_tile[:],
                )
            if nxt is not None:
                nxt[1]()
    nc.compile()
    return nc


def _get_nc():
    if "nc" not in _NC_CACHE:
        _NC_CACHE["nc"] = _build_nc()
    return _NC_CACHE["nc"]


def _impulse_response(b, a, n):
    b = np.asarray(b, np.float64)
    a = np.asarray(a, np.float64)
    b = b / a[0]
    a = a / a[0]
    h = np.zeros(n, np.float64)
    for t in range(n):
        acc = b[t] if t < len(b) else 0.0
        kmax = min(len(a) - 1, t)
        for k in range(1, kmax + 1):
            acc -= a[k] * h[t - k]
        h[t] = acc
    return h


def _build_slabs(h):
    """slab_p[i, j] = h[(c0_p + j) + W - 128 p - i], zero outside [0, W)."""
    i = np.arange(128)
    slabs = np.zeros((128, SLAB_COLS), np.float32)
    for p, ((c0, w), off) in enumerate(zip(SLAB_SPECS, SLAB_OFFS)):
        n = c0 + np.arange(w)
        d = n[None, :] + W - 128 * p - i[:, None]
        valid = (d >= 0) & (d < W)
        vals = np.where(valid, h[np.clip(d, 0, W - 1)], 0.0)
        slabs[:, off:off + w] = vals.astype(np.float32)
    return slabs


def kernel_with_results(x, b, a, trace=False):
    from concourse.bass_utils import run_bass_kernel_spmd

    x = np.asarray(x, np.float32)
    h = _impulse_response(np.asarray(b), np.asarray(a), W)
    slabs = _build_slabs(h)
    ident = np.eye(128, dtype=np.float32)

    xs = np.asarray(x).reshape(NSIG, T)
    in_maps = [
        {
            "x": np.ascontiguousarray(xs[c * SIG_PER_CORE:(c + 1) * SIG_PER_CORE]),
            "slabs": slabs,
            "ident": ident,
        }
        for c in range(N_CORES)
    ]
    nc = _get_nc()
    res = run_bass_kernel_spmd(nc, in_maps, core_ids=list(range(N_CORES)), trace=trace)
    y = np.concatenate([res.results[c]["y"] for c in range(N_CORES)], axis=0)
    return y.reshape(B, C, T).astype(np.float32), res


def kernel(x, b, a):
    os.environ.setdefault("BASS_NEVER_TRACE", "1")
    y, _ = kernel_with_results(x, b, a, trace=False)
    return y

